# revision 2
# baseline (speedup 1.0000x reference)
"""Attention-pooling kernel for TRN2 (8 NeuronCores, data-parallel over batch).

Computes, per batch b:
    scores = seeds @ x[b].T          # [M, S]
    weights = softmax(scores, -1)
    out[b] = weights @ x[b]          # [M, D]

Sharding: batch B=32 split 4-per-core across 8 cores; seeds replicated.

Per-core pipeline (all bf16 on-chip, f32 PSUM accumulation):
  - SWDGE cast-DMA loads x tiles HBM f32 -> SBUF bf16 (cast rides the DMA).
  - PE transposes x 128x128 blocks (bf16, FWL weight loads) -> psum -> DVE
    copies to SBUF as x^T chunks.
  - scores: 4 accumulating matmuls lhsT=seedsT chunk [128,16], rhs=xT [128,512].
  - exp on ACT straight out of PSUM, with fused accum_out row-sums
    (no max subtraction: scores = seeds.x are bounded ~|8|, exp is safe in f32).
  - PE transposes exp [16,128] -> expT [128,16]; pooled matmul accumulates
    psum_pool[16,512] over the whole batch with lhsT=expT, rhs=x tile.
  - batch end: recip(sum) on DVE, scale pooled, DMA out f32.
"""

import os
from contextlib import ExitStack

import numpy as np

import concourse.bass as bass
import concourse.mybir as mybir
import concourse.tile as tile
from concourse import bacc
from concourse.bass_utils import run_bass_kernel_spmd
from concourse.masks import make_identity

N_CORES = 8
B, S, D, M = 32, 8192, 512, 16
S_MACRO = 512          # s rows per macro-tile
T_SUB = S_MACRO // 128  # 128-row subtiles per macro-tile
DC = D // 128           # 128-col d chunks

f32 = mybir.dt.float32
bf16 = mybir.dt.bfloat16


def kernel_body(tc, out_ap, x_ap, seeds_ap, b_loc, s):
    nc = tc.nc
    n_mac = s // S_MACRO
    with ExitStack() as ctx:
        const = ctx.enter_context(tc.tile_pool(name="const", bufs=1))
        xp = ctx.enter_context(tc.tile_pool(name="xp", bufs=3))
        xtp = ctx.enter_context(tc.tile_pool(name="xtp", bufs=3))
        ep = ctx.enter_context(tc.tile_pool(name="ep", bufs=3))
        etp = ctx.enter_context(tc.tile_pool(name="etp", bufs=3))
        statp = ctx.enter_context(tc.tile_pool(name="statp", bufs=4))
        outp = ctx.enter_context(tc.tile_pool(name="outp", bufs=2))
        ps_xt = ctx.enter_context(tc.tile_pool(name="ps_xt", bufs=3, space="PSUM"))
        ps_sc = ctx.enter_context(tc.tile_pool(name="ps_sc", bufs=2, space="PSUM"))
        ps_et = ctx.enter_context(tc.tile_pool(name="ps_et", bufs=2, space="PSUM"))
        ps_pl = ctx.enter_context(tc.tile_pool(name="ps_pl", bufs=1, space="PSUM"))

        ident = const.tile([128, 128], bf16)
        make_identity(nc, ident)

        # seeds -> bf16 -> seedsT [d, m] chunks, [128, DC*M] (dc-major)
        seeds_bf = const.tile([M, D], bf16)
        nc.gpsimd.dma_start(out=seeds_bf[:], in_=seeds_ap)
        ps_st = ps_et.tile([128, DC * M], bf16, tag="et")
        for dc in range(DC):
            nc.tensor.transpose(
                ps_st[:, dc * M:(dc + 1) * M],
                seeds_bf[:, dc * 128:(dc + 1) * 128],
                ident[:M, :M],
            )
        seedsT = const.tile([128, DC * M], bf16)
        nc.vector.tensor_copy(seedsT[:], ps_st[:])

        # x view: [b, n, p, t, d] with s = n*S_MACRO + t*128 + p
        x_r = x_ap.rearrange("b (n t p) d -> b n p t d", t=T_SUB, p=128)

        for bb in range(b_loc):
            sums = statp.tile([M, n_mac], f32, tag="sums")
            pool_ps = ps_pl.tile([M, D], f32, tag="pl")
            for n in range(n_mac):
                x_bf = xp.tile([128, T_SUB, D], bf16, tag="x")
                nc.gpsimd.dma_start(out=x_bf[:], in_=x_r[bb, n])

                # transpose x: 16 blocks of 128x128 -> xT [128, (dc, s)] bf16
                xt_sb = xtp.tile([128, DC, S_MACRO], bf16, tag="xt")
                n_ps = (DC + 1) // 2
                for ph in range(n_ps):  # 2 psum banks, 2 dc chunks each
                    xt_ps = ps_xt.tile([128, 2 * S_MACRO], bf16, tag="xt")
                    for dch in range(2):
                        dc = ph * 2 + dch
                        for t in range(T_SUB):
                            nc.tensor.transpose(
                                xt_ps[:, dch * S_MACRO + t * 128:
                                      dch * S_MACRO + (t + 1) * 128],
                                x_bf[:, t, dc * 128:(dc + 1) * 128],
                                ident[:],
                            )
                    nc.vector.tensor_copy(
                        xt_sb[:, ph * 2:(ph + 1) * 2, :], xt_ps[:]
                    )

                # scores: [M, S_MACRO] psum accumulated over DC chunks
                sc_ps = ps_sc.tile([M, S_MACRO], f32, tag="sc")
                for dc in range(DC):
                    nc.tensor.matmul(
                        sc_ps[:],
                        lhsT=seedsT[:, dc * M:(dc + 1) * M],
                        rhs=xt_sb[:, dc, :],
                        start=(dc == 0),
                        stop=(dc == DC - 1),
                    )

                # exp with fused row-sum
                e_bf = ep.tile([M, S_MACRO], bf16, tag="e")
                nc.scalar.activation(
                    e_bf[:], sc_ps[:], mybir.ActivationFunctionType.Exp,
                    accum_out=sums[:, n:n + 1],
                )

                # expT [s, m] per subtile
                et_ps = ps_et.tile([128, T_SUB * M], bf16, tag="et")
                for t in range(T_SUB):
                    nc.tensor.transpose(
                        et_ps[:, t * M:(t + 1) * M],
                        e_bf[:, t * 128:(t + 1) * 128],
                        ident[:M, :M],
                    )
                et_sb = etp.tile([128, T_SUB * M], bf16, tag="et")
                nc.vector.tensor_copy(et_sb[:], et_ps[:])

                # pooled accumulation over the whole batch
                for t in range(T_SUB):
                    nc.tensor.matmul(
                        pool_ps[:],
                        lhsT=et_sb[:, t * M:(t + 1) * M],
                        rhs=x_bf[:, t, :],
                        start=(n == 0 and t == 0),
                        stop=(n == n_mac - 1 and t == T_SUB - 1),
                    )

            total = statp.tile([M, 1], f32, tag="tot")
            nc.vector.reduce_sum(total[:], sums[:], axis=mybir.AxisListType.X)
            recip = statp.tile([M, 1], f32, tag="rec")
            nc.vector.reciprocal(recip[:], total[:])
            o_sb = outp.tile([M, D], f32, tag="o")
            nc.vector.tensor_scalar_mul(o_sb[:], pool_ps[:], recip[:])
            nc.sync.dma_start(out=out_ap[bb], in_=o_sb[:])


def build_bass(b_loc, s):
    nc = bacc.Bacc(
        "TRN2", target_bir_lowering=False, debug=False, num_devices=N_CORES
    )
    x_d = nc.dram_tensor("x", [b_loc, s, D], f32, kind="ExternalInput")
    seeds_d = nc.dram_tensor("seeds", [M, D], f32, kind="ExternalInput")
    out_d = nc.dram_tensor("out", [b_loc, M, D], f32, kind="ExternalOutput")
    with tile.TileContext(nc) as tc:
        kernel_body(tc, out_d.ap(), x_d.ap(), seeds_d.ap(), b_loc, s)
    nc.compile()
    return nc


_cached = {}


def get_nc(b_loc, s):
    key = (b_loc, s)
    if key not in _cached:
        _cached[key] = build_bass(b_loc, s)
    return _cached[key]


def kernel(x, seeds, trace=False):
    assert x.shape == (B, S, D) and seeds.shape == (M, D)
    x = np.asarray(x, dtype=np.float32)
    seeds = np.asarray(seeds, dtype=np.float32)
    b_loc = B // N_CORES
    nc = get_nc(b_loc, S)
    in_maps = [
        {
            "x": np.ascontiguousarray(x[i * b_loc:(i + 1) * b_loc]),
            "seeds": seeds,
        }
        for i in range(N_CORES)
    ]
    res = run_bass_kernel_spmd(
        nc, in_maps, core_ids=list(range(N_CORES)), trace=trace
    )
    out = np.concatenate([r["out"] for r in res.results], axis=0)
    if trace:
        kernel.last_result = res
    return out.astype(np.float32)


kernel.last_result = None
